# revision 1
# baseline (speedup 1.0000x reference)
"""MegablockMoE kernel for 8 Trainium2 NeuronCores.

Strategy (per sharding hint): expert-parallel. The router + token
dispatch/combine permutations (pure index bookkeeping) run on host as the
shard/unshard step; each of the 8 cores owns one expert and runs the two big
GEMMs (gelu(xg @ w1[e]) @ w2[e], 34.4 GFLOP/core) in bf16 with fp32 PSUM
accumulation, weights resident in SBUF, hT intermediate never leaving chip.

Device kernel (identical NEFF on all 8 cores, SPMD over experts):
    in : xgT [D, C] bf16   -- gathered tokens for this expert, transposed
         w1  [D, DFF] bf16, w2 [DFF, D] bf16
    mid: hT  [DFF, c_tile] bf16 = gelu(w1.T @ xgT)   (exact erf gelu, SBUF)
    out: yT  [D, C] f32    = w2.T @ hT
"""

import numpy as np
import ml_dtypes

import concourse.mybir as mybir
import concourse.tile as tile
from concourse import bacc
from concourse.bass_utils import run_bass_kernel_spmd

B, S, D = 4, 2048, 1024
E, K, DFF = 8, 2, 4096
T = B * S
C = K * T // E  # 2048 expert capacity
BF16 = ml_dtypes.bfloat16
N_CORES = 8

KO1, KO2 = D // 128, DFF // 128
W1_CH = 8            # f-chunks of w1 (separate tiles -> fine-grained DMA deps)
W1_F = DFF // W1_CH  # 512
W2_CH = 8            # o-chunks of w2
W2_O = KO2 // W2_CH  # 4

_NC = None


def _build_nc(c_tile=256, psum_bufs=4, ht_bufs=2, y_bufs=2, xg_bufs=2,
              n_iters=1, debug=True):
    nc = bacc.Bacc(None, target_bir_lowering=False, debug=debug)
    xgT = nc.dram_tensor("xgT", [D, C], mybir.dt.bfloat16, kind="ExternalInput")
    w1 = nc.dram_tensor("w1", [D, DFF], mybir.dt.bfloat16, kind="ExternalInput")
    w2 = nc.dram_tensor("w2", [DFF, D], mybir.dt.bfloat16, kind="ExternalInput")
    yT = nc.dram_tensor("yT", [D, C], mybir.dt.float32, kind="ExternalOutput")

    xgT_v = xgT.rearrange("(o p) c -> p o c", p=128)
    w1_v = w1.rearrange("(o p) f -> p o f", p=128)
    w2_v = w2.rearrange("(o p) d -> p o d", p=128)
    yT_v = yT.rearrange("(o p) c -> p o c", p=128)
    n_ct = C // c_tile

    with tile.TileContext(nc) as tc:
        with (
            tc.tile_pool(name="wpool", bufs=1) as wpool,
            tc.tile_pool(name="xpool", bufs=xg_bufs) as xpool,
            tc.tile_pool(name="hpool", bufs=ht_bufs) as hpool,
            tc.tile_pool(name="ypool", bufs=y_bufs) as ypool,
            tc.tile_pool(name="ps1", bufs=psum_bufs, space="PSUM") as ps1,
            tc.tile_pool(name="ps2", bufs=psum_bufs, space="PSUM") as ps2,
        ):
            # first xg tile before weights: small and needed immediately
            xg_tiles = {}
            if n_iters == 1:
                xg_tiles[0] = xpool.tile([128, KO1, c_tile], mybir.dt.bfloat16,
                                         tag="xg", name="xg0")
                nc.sync.dma_start(xg_tiles[0][:], xgT_v[:, :, 0:c_tile])

            w1_tiles = []
            for ch in range(W1_CH):
                wt = wpool.tile([128, KO1, W1_F], mybir.dt.bfloat16,
                                tag=f"w1_{ch}", name=f"w1t{ch}")
                nc.sync.dma_start(wt[:], w1_v[:, :, ch * W1_F : (ch + 1) * W1_F])
                w1_tiles.append(wt)
            w2_tiles = []
            for ch in range(W2_CH):
                wt = wpool.tile([128, W2_O, D], mybir.dt.bfloat16,
                                tag=f"w2_{ch}", name=f"w2t{ch}")
                nc.sync.dma_start(wt[:], w2_v[:, ch * W2_O : (ch + 1) * W2_O, :])
                w2_tiles.append(wt)

            def w1_ap(o, f):
                ch, r = divmod(f, W1_F // 128)
                return w1_tiles[ch][:, o, r * 128 : (r + 1) * 128]

            def w2_ap(f, g):
                ch, r = divmod(f, W2_O)
                return w2_tiles[ch][:, r, g * 128 : (g + 1) * 128]

            def evict_y(y_sb, g, psum):
                # split evictions across DVE and ACT so neither engine's
                # latency throttles the PE stream
                if g % 2 == 1:
                    nc.scalar.copy(y_sb[:, g, :], psum[:])
                else:
                    nc.vector.tensor_copy(y_sb[:, g, :], psum[:])

            def body(_=None):
                for t in range(n_ct):
                    cs = slice(t * c_tile, (t + 1) * c_tile)
                    if t not in xg_tiles:
                        xg_tiles[t] = xpool.tile(
                            [128, KO1, c_tile], mybir.dt.bfloat16, tag="xg",
                            name=f"xg{t}",
                        )
                        nc.sync.dma_start(xg_tiles[t][:], xgT_v[:, :, cs])
                    xg_sb = xg_tiles[t]

                    hT_sb = hpool.tile([128, KO2, c_tile], mybir.dt.bfloat16,
                                       tag="hT")
                    y_sb = ypool.tile([128, KO1, c_tile], mybir.dt.float32,
                                      tag="y")
                    # pass A: mm1(f) interleaved with mm2 for g in 0..3 —
                    # mm2's f-step consumes hT[f] right after its eviction,
                    # removing the mm1->mm2 phase boundary. 4 ps1 bufs +
                    # 4 accumulating mm2 psums = all 8 PSUM banks.
                    ps2g = [ps2.tile([128, c_tile], mybir.dt.float32,
                                     tag="p2", name=f"p2a{t}_{g}")
                            for g in range(4)]
                    for f in range(KO2):
                        psum = ps1.tile([128, c_tile], mybir.dt.float32,
                                        tag="p1")
                        for o in range(KO1):
                            nc.tensor.matmul(
                                psum[:], w1_ap(o, f), xg_sb[:, o, :],
                                start=(o == 0), stop=(o == KO1 - 1),
                            )
                        nc.scalar.activation(
                            hT_sb[:, f, :], psum[:],
                            mybir.ActivationFunctionType.Gelu,
                        )
                        for g in range(4):
                            nc.tensor.matmul(
                                ps2g[g][:], w2_ap(f, g), hT_sb[:, f, :],
                                start=(f == 0), stop=(f == KO2 - 1),
                            )
                    for g in range(4):
                        evict_y(y_sb, g, ps2g[g])
                    # pass B: mm2 for g in 4..7 (hT complete by now)
                    ps2h = [ps2.tile([128, c_tile], mybir.dt.float32,
                                     tag="p2", name=f"p2b{t}_{g}")
                            for g in range(4)]
                    for f in range(KO2):
                        for g in range(4):
                            nc.tensor.matmul(
                                ps2h[g][:], w2_ap(f, g + 4), hT_sb[:, f, :],
                                start=(f == 0), stop=(f == KO2 - 1),
                            )
                    for g in range(4):
                        evict_y(y_sb, g + 4, ps2h[g])
                    nc.sync.dma_start(yT_v[:, :, cs], y_sb[:])

            if n_iters == 1:
                body()
            else:
                with tc.For_i(0, n_iters, 1):
                    body()
    nc.compile()
    return nc


def _get_nc():
    global _NC
    if _NC is None:
        _NC = _build_nc()
    return _NC


def _route(x, wr):
    """Replicates the reference router exactly (fp32 numpy)."""
    xt = np.transpose(x, (1, 0, 2)).reshape(T, D)  # [T, D] fp32
    logits = xt.astype(np.float32) @ wr.astype(np.float32)  # [T, E]
    m = logits.max(axis=-1, keepdims=True)
    p = np.exp(logits - m, dtype=np.float32)
    p /= p.sum(axis=-1, keepdims=True)
    top1 = np.argmax(p, axis=-1)
    p_masked = p.copy()
    p_masked[np.arange(T), top1] = -np.inf
    top2 = np.argmax(p_masked, axis=-1)
    eidx = np.stack([top1, top2], axis=1)  # [T, K]
    ew = np.take_along_axis(p, eidx, axis=1).astype(np.float32)  # [T, K]

    flat_e = eidx.reshape(-1)
    order = np.argsort(flat_e, kind="stable")
    sorted_e = flat_e[order]
    hist = np.bincount(flat_e, minlength=E)
    starts = np.cumsum(hist) - hist
    pos = np.arange(T * K) - starts[sorted_e]
    keep = pos < C
    slot = np.where(keep, sorted_e * C + pos, E * C)
    token = order // K
    return xt, ew, order, keep, slot, token


def _make_in_maps(x, wr, w1, w2):
    xt, ew, order, keep, slot, token = _route(x, wr)
    slot_token = np.zeros(E * C, np.int64)
    slot_token[slot[keep]] = token[keep]
    xT_bf = np.ascontiguousarray(xt.T.astype(BF16))  # [D, T]
    in_maps = []
    for e in range(E):
        idx = slot_token[e * C : (e + 1) * C]
        in_maps.append(
            {
                "xgT": np.ascontiguousarray(xT_bf[:, idx]),
                "w1": np.ascontiguousarray(w1[e].astype(BF16)),
                "w2": np.ascontiguousarray(w2[e].astype(BF16)),
            }
        )
    return in_maps, (ew, order, keep, slot)


def kernel(x, wr, w1, w2):
    nc = _get_nc()
    in_maps, (ew, order, keep, slot) = _make_in_maps(x, wr, w1, w2)

    res = run_bass_kernel_spmd(nc, in_maps, core_ids=list(range(N_CORES)))

    # --- combine: weighted scatter back to tokens ---
    Y = np.empty((E * C, D), np.float32)
    for e in range(E):
        Y[e * C : (e + 1) * C] = res.results[e]["yT"].T

    inv = np.empty(T * K, np.int64)
    inv[order] = np.arange(T * K)
    slot_tk = slot[inv].reshape(T, K)
    keep_tk = keep[inv].reshape(T, K)

    out_flat = np.zeros((T, D), np.float32)
    for k in range(K):
        sl = np.clip(slot_tk[:, k], 0, E * C - 1)
        contrib = Y[sl] * ew[:, k : k + 1]
        contrib[~keep_tk[:, k]] = 0.0
        out_flat += contrib
    return np.ascontiguousarray(
        out_flat.reshape(S, B, D).transpose(1, 0, 2)
    ).astype(np.float32)


# ---------------------------------------------------------------------------
# Benchmark helper (used by test.py; not part of the grading contract).
# ---------------------------------------------------------------------------


def make_bench(in_maps):
    import jax
    from jax.experimental.shard_map import shard_map
    from jax.sharding import Mesh, PartitionSpec, NamedSharding
    from concourse.bass2jax import (
        _bass_exec_p,
        install_neuronx_cc_hook,
        partition_id_tensor,
    )

    nc = _NC if _NC is not None else _get_nc()
    install_neuronx_cc_hook()
    partition_name = nc.partition_id_tensor.name if nc.partition_id_tensor else None

    in_names, out_names, out_avals, zero_outs = [], [], [], []
    for alloc in nc.m.functions[0].allocations:
        if not isinstance(alloc, mybir.MemoryLocationSet):
            continue
        name = alloc.memorylocations[0].name
        if alloc.kind == "ExternalInput":
            if name != partition_name:
                in_names.append(name)
        elif alloc.kind == "ExternalOutput":
            shape = tuple(alloc.tensor_shape)
            dtype = mybir.dt.np(alloc.dtype)
            out_avals.append(jax.core.ShapedArray(shape, dtype))
            zero_outs.append(np.zeros(shape, dtype))
            out_names.append(name)
    n_params = len(in_names)
    all_in_names = list(in_names) + list(out_names)
    if partition_name is not None:
        all_in_names.append(partition_name)
    if nc.dbg_addr is not None:
        dbg_zero = np.zeros((1, 2), np.uint32)
        in_maps = [{**m, nc.dbg_addr.name: dbg_zero} for m in in_maps]

    def _body(*args):
        operands = list(args)
        if partition_name is not None:
            operands.append(partition_id_tensor())
        outs = _bass_exec_p.bind(
            *operands,
            out_avals=tuple(out_avals),
            in_names=tuple(all_in_names),
            out_names=tuple(out_names),
            lowering_input_output_aliases=(),
            sim_require_finite=True,
            sim_require_nnan=True,
            nc=nc,
        )
        return tuple(outs)

    devices = jax.devices()[:N_CORES]
    mesh = Mesh(np.asarray(devices), ("core",))
    n_outs = len(out_names)
    in_specs = (PartitionSpec("core"),) * (n_params + n_outs)
    out_specs = (PartitionSpec("core"),) * n_outs
    fn = jax.jit(
        shard_map(_body, mesh=mesh, in_specs=in_specs, out_specs=out_specs,
                  check_rep=False),
        keep_unused=True,
    )
    concat_in = [
        np.concatenate([np.asarray(in_maps[c][name]) for c in range(N_CORES)],
                       axis=0)
        for name in in_names
    ]
    concat_zeros = [
        np.zeros((N_CORES * z.shape[0], *z.shape[1:]), z.dtype)
        for z in zero_outs
    ]
    shard = NamedSharding(mesh, PartitionSpec("core"))
    args = [jax.device_put(a, shard) for a in concat_in + concat_zeros]
    return fn, args, out_names


def benchmark(in_maps, iters=20, warmup=3):
    import time
    import jax

    fn, args, out_names = make_bench(in_maps)
    for _ in range(warmup):
        out = fn(*args)
        jax.block_until_ready(out)
    times = []
    for _ in range(iters):
        t0 = time.perf_counter()
        out = fn(*args)
        jax.block_until_ready(out)
        times.append(time.perf_counter() - t0)
    return min(times), sorted(times)[len(times) // 2], out



# revision 8
# speedup vs baseline: 2.0766x; 2.0766x over previous
"""MegablockMoE kernel for 8 Trainium2 NeuronCores.

Strategy (per sharding hint): expert-parallel. The router + token
dispatch/combine permutations (pure index bookkeeping) run on host as the
shard/unshard step; each of the 8 cores owns one expert and runs the two big
GEMMs (gelu(xg @ w1[e]) @ w2[e], 34.4 GFLOP/core) with weights resident in
SBUF and the hT intermediate never leaving chip.

Precision split: the kernel is PE-bound at the bf16 roofline, so the Q=512
slots with the smallest routing weights (per expert) run in fp8-e4m3 with
DoubleRow matmuls (2 fp8 MACs/cell/cycle). Their output error is attenuated
by the small expert weight: measured end-to-end rel-L2 ~1.2e-2 vs the 2e-2
gate. Host sorts each expert's capacity buffer by routing weight
(descending); cols 0..1535 run bf16, cols 1536..2047 run fp8.

Device kernel (identical NEFF on all 8 cores, SPMD over experts):
    in : xgT  [D, 1536] bf16, xg8T [D, 512] fp8 (x16)
         w1/w2 [.,.] bf16, w18/w28 fp8 (x4096, streamed)
    out: yT [D, C] bf16
"""

import numpy as np
import ml_dtypes

import concourse.mybir as mybir
import concourse.tile as tile
from concourse import bacc
from concourse.bass_utils import run_bass_kernel_spmd

B, S, D = 4, 2048, 1024
E, K, DFF = 8, 2, 4096
T = B * S
C = K * T // E  # 2048 expert capacity
BF16 = ml_dtypes.bfloat16
F8 = ml_dtypes.float8_e4m3  # == mybir.dt.np(float8e4)
N_CORES = 8

Q = 512           # fp8 slots per expert (smallest routing weight)
CB = C - Q        # bf16 slots
SX = 16.0         # fp8 scale for activations x
SW = 4096.0       # fp8 scale for weights

KO1, KO2 = D // 128, DFF // 128
W1_CH = 8            # f-chunks of w1 (separate tiles -> fine-grained DMA deps)
W1_F = DFF // W1_CH  # 512
W2_CH = 8            # o-chunks of w2
W2_O = KO2 // W2_CH  # 4

_NC = None


def _build_nc(c_tile=512, ps1_bufs=4, ps2_bufs=4, ht_bufs=1, y_bufs=2,
              xg_bufs=2, y_dt="bf16", fp8=True, q3_mm2=True, n_iters=1,
              debug=True):
    f32 = mybir.dt.float32
    f8 = mybir.dt.float8e4
    y_mydt = mybir.dt.bfloat16 if y_dt == "bf16" else mybir.dt.float32
    DR = mybir.MatmulPerfMode.DoubleRow
    cb = CB if fp8 else C

    nc = bacc.Bacc(None, target_bir_lowering=False, debug=debug)
    xgT = nc.dram_tensor("xgT", [D, cb], mybir.dt.bfloat16, kind="ExternalInput")
    w1 = nc.dram_tensor("w1", [D, DFF], mybir.dt.bfloat16, kind="ExternalInput")
    w2 = nc.dram_tensor("w2", [DFF, D], mybir.dt.bfloat16, kind="ExternalInput")
    if fp8:
        xg8T = nc.dram_tensor("xg8T", [D, Q], f8, kind="ExternalInput")
        w18 = nc.dram_tensor("w18", [D, DFF], f8, kind="ExternalInput")
        w28 = nc.dram_tensor("w28", [DFF, D], f8, kind="ExternalInput")
    yT = nc.dram_tensor("yT", [D, C], y_mydt, kind="ExternalOutput")

    xgT_v = xgT.rearrange("(o p) c -> p o c", p=128)
    w1_v = w1.rearrange("(o p) f -> p o f", p=128)
    w2_v = w2.rearrange("(o p) d -> p o d", p=128)
    yT_v = yT.rearrange("(o p) c -> p o c", p=128)
    if fp8:
        xg8T_v = xg8T.rearrange("(o p) c -> p o c", p=128)
        w18_v = w18.rearrange("(o2 i p) f -> p o2 i f", p=128, i=2)
        w28_v = w28.rearrange("(f2 i p) d -> p f2 i d", p=128, i=2)
    n_ct = cb // c_tile

    with tile.TileContext(nc) as tc:
        with (
            tc.tile_pool(name="wpool", bufs=1) as wpool,
            tc.tile_pool(name="w18p", bufs=4) as w18p,
            tc.tile_pool(name="w28p", bufs=4) as w28p,
            tc.tile_pool(name="xpool", bufs=xg_bufs) as xpool,
            tc.tile_pool(name="hpool", bufs=ht_bufs) as hpool,
            tc.tile_pool(name="ypool", bufs=y_bufs) as ypool,
            tc.tile_pool(name="ps1", bufs=ps1_bufs, space="PSUM") as ps1,
            tc.tile_pool(name="ps2", bufs=ps2_bufs, space="PSUM") as ps2,
        ):
            # first xg tile before weights: small and needed immediately
            xg_tiles = {}
            if n_iters == 1:
                xg_tiles[0] = xpool.tile([128, KO1, c_tile], mybir.dt.bfloat16,
                                         tag="xg", name="xg0")
                nc.sync.dma_start(xg_tiles[0][:], xgT_v[:, :, 0:c_tile])

            w1_tiles = []
            for ch in range(W1_CH):
                wt = wpool.tile([128, KO1, W1_F], mybir.dt.bfloat16,
                                tag=f"w1_{ch}", name=f"w1t{ch}")
                nc.sync.dma_start(wt[:], w1_v[:, :, ch * W1_F : (ch + 1) * W1_F])
                w1_tiles.append(wt)
            w2_tiles = []
            for ch in range(W2_CH):
                wt = wpool.tile([128, W2_O, D], mybir.dt.bfloat16,
                                tag=f"w2_{ch}", name=f"w2t{ch}")
                nc.sync.dma_start(wt[:], w2_v[:, ch * W2_O : (ch + 1) * W2_O, :])
                w2_tiles.append(wt)

            def w1_ap(o, f):
                ch, r = divmod(f, W1_F // 128)
                return w1_tiles[ch][:, o, r * 128 : (r + 1) * 128]

            def w2_ap(f, g):
                ch, r = divmod(f, W2_O)
                return w2_tiles[ch][:, r, g * 128 : (g + 1) * 128]

            def evict_y(y_sb, g, psum, scale=None):
                # split evictions across DVE and ACT so neither engine's
                # latency throttles the PE stream
                if scale is None:
                    if g % 2 == 1:
                        nc.scalar.copy(y_sb[:, g, :], psum[:])
                    else:
                        nc.vector.tensor_copy(y_sb[:, g, :], psum[:])
                else:
                    if g % 2 == 1:
                        nc.scalar.activation(
                            y_sb[:, g, :], psum[:],
                            mybir.ActivationFunctionType.Copy, scale=scale)
                    else:
                        nc.vector.tensor_scalar_mul(y_sb[:, g, :], psum[:],
                                                    scale)

            def get_w28(pool_cache, it, f2, half):
                key = (f2, half)
                if key not in pool_cache:
                    wt = w28p.tile([128, 2, 512], f8, tag="w28",
                                   name=f"w28c{it}_{f2}_{half}")
                    nc.sync.dma_start(
                        wt[:],
                        w28_v[:, f2, :, half * 512 : (half + 1) * 512])
                    pool_cache[key] = wt
                return pool_cache[key]

            def bf16_tile(t, mm2_fp8=False):
                cs = slice(t * c_tile, (t + 1) * c_tile)
                if t not in xg_tiles:
                    xg_tiles[t] = xpool.tile(
                        [128, KO1, c_tile], mybir.dt.bfloat16, tag="xg",
                        name=f"xg{t}",
                    )
                    nc.sync.dma_start(xg_tiles[t][:], xgT_v[:, :, cs])
                xg_sb = xg_tiles[t]

                h_dt = f8 if mm2_fp8 else mybir.dt.bfloat16
                hT_sb = hpool.tile([128, KO2, c_tile], h_dt, tag="hT",
                                   name=f"hT{t}")
                y_sb = ypool.tile([128, KO1, c_tile], y_mydt, tag="y",
                                  name=f"y{t}")
                w28c = {}
                ev_scale = (1.0 / SW) if mm2_fp8 else None

                def mm2_step(accums, f2, g, half):
                    w28t = get_w28(w28c, f"q{t}", f2, half)
                    nc.tensor.matmul(
                        accums[g][:], w28t[:, :, g * 128 : (g + 1) * 128],
                        hT_sb[:, 2 * f2 : 2 * f2 + 2, :],
                        start=(f2 == 0), stop=(f2 == KO2 // 2 - 1),
                        perf_mode=DR,
                    )

                # pass A: mm1(f) interleaved with mm2 for g in 0..3 —
                # mm2's f-step consumes hT[f] right after its eviction,
                # removing the mm1->mm2 phase boundary.
                ps2g = [ps2.tile([128, c_tile], f32, tag="p2",
                                 name=f"p2a{t}_{g}")
                        for g in range(4)]
                for f in range(KO2):
                    psum = ps1.tile([128, c_tile], f32, tag="p1")
                    for o in range(KO1):
                        nc.tensor.matmul(
                            psum[:], w1_ap(o, f), xg_sb[:, o, :],
                            start=(o == 0), stop=(o == KO1 - 1),
                        )
                    nc.scalar.activation(
                        hT_sb[:, f, :], psum[:],
                        mybir.ActivationFunctionType.Gelu,
                    )
                    if mm2_fp8:
                        if f % 2 == 1:
                            for g in range(4):
                                mm2_step(ps2g, f // 2, g, 0)
                    else:
                        for g in range(4):
                            nc.tensor.matmul(
                                ps2g[g][:], w2_ap(f, g), hT_sb[:, f, :],
                                start=(f == 0), stop=(f == KO2 - 1),
                            )
                for g in range(4):
                    evict_y(y_sb, g, ps2g[g], scale=ev_scale)
                # pass B: mm2 for g in 4..7 (hT complete by now)
                ps2h = [ps2.tile([128, c_tile], f32, tag="p2",
                                 name=f"p2b{t}_{g}")
                        for g in range(4)]
                if mm2_fp8:
                    for f2 in range(KO2 // 2):
                        w28t = get_w28(w28c, f"q{t}", f2, 1)
                        for g in range(4):
                            nc.tensor.matmul(
                                ps2h[g][:],
                                w28t[:, :, g * 128 : (g + 1) * 128],
                                hT_sb[:, 2 * f2 : 2 * f2 + 2, :],
                                start=(f2 == 0), stop=(f2 == KO2 // 2 - 1),
                                perf_mode=DR,
                            )
                else:
                    for f in range(KO2):
                        for g in range(4):
                            nc.tensor.matmul(
                                ps2h[g][:], w2_ap(f, g + 4), hT_sb[:, f, :],
                                start=(f == 0), stop=(f == KO2 - 1),
                            )
                for g in range(4):
                    evict_y(y_sb, g + 4, ps2h[g], scale=ev_scale)
                nc.sync.dma_start(yT_v[:, :, cs], y_sb[:])

            def fp8_tile(it=""):
                # slots CB..C-1 in fp8 with DoubleRow (256-contraction MMs)
                xg8_sb = xpool.tile([128, KO1, Q], f8, tag="xg",
                                    name=f"xg8{it}")
                nc.sync.dma_start(xg8_sb[:], xg8T_v[:, :, :])
                h8_sb = hpool.tile([128, KO2, Q], f8, tag="hT",
                                   name=f"h8{it}")
                y_sb = ypool.tile([128, KO1, Q], y_mydt, tag="y",
                                  name=f"y8{it}")
                w18c = {}
                w28c = {}

                def get_w18(f):
                    if f not in w18c:
                        wt = w18p.tile([128, 4, 2, 128], f8, tag="w18",
                                       name=f"w18c{it}_{f}")
                        nc.sync.dma_start(
                            wt[:], w18_v[:, :, :, f * 128 : (f + 1) * 128])
                        w18c[f] = wt
                    return w18c[f]

                def get_w28(f2, half):
                    key = (f2, half)
                    if key not in w28c:
                        wt = w28p.tile([128, 2, 512], f8, tag="w28",
                                       name=f"w28c{it}_{f2}_{half}")
                        nc.sync.dma_start(
                            wt[:],
                            w28_v[:, f2, :, half * 512 : (half + 1) * 512])
                        w28c[key] = wt
                    return w28c[key]

                ps2g = [ps2.tile([128, Q], f32, tag="p2", name=f"p2f8a{it}_{g}")
                        for g in range(4)]
                for f in range(KO2):
                    psum = ps1.tile([128, Q], f32, tag="p1")
                    w18t = get_w18(f)
                    for o2 in range(4):
                        nc.tensor.matmul(
                            psum[:], w18t[:, o2, :, :],
                            xg8_sb[:, 2 * o2 : 2 * o2 + 2, :],
                            start=(o2 == 0), stop=(o2 == 3), perf_mode=DR,
                        )
                    nc.scalar.activation(
                        h8_sb[:, f, :], psum[:],
                        mybir.ActivationFunctionType.Gelu,
                        scale=1.0 / (SX * SW),
                    )
                    if f % 2 == 1:
                        f2 = f // 2
                        w28t = get_w28(f2, 0)
                        for g in range(4):
                            nc.tensor.matmul(
                                ps2g[g][:], w28t[:, :, g * 128 : (g + 1) * 128],
                                h8_sb[:, 2 * f2 : 2 * f2 + 2, :],
                                start=(f2 == 0), stop=(f2 == 15), perf_mode=DR,
                            )
                for g in range(4):
                    evict_y(y_sb, g, ps2g[g], scale=1.0 / SW)
                ps2h = [ps2.tile([128, Q], f32, tag="p2", name=f"p2f8b{it}_{g}")
                        for g in range(4)]
                for f2 in range(KO2 // 2):
                    w28t = get_w28(f2, 1)
                    for g in range(4):
                        nc.tensor.matmul(
                            ps2h[g][:], w28t[:, :, g * 128 : (g + 1) * 128],
                            h8_sb[:, 2 * f2 : 2 * f2 + 2, :],
                            start=(f2 == 0), stop=(f2 == 15), perf_mode=DR,
                        )
                for g in range(4):
                    evict_y(y_sb, g + 4, ps2h[g], scale=1.0 / SW)
                nc.sync.dma_start(yT_v[:, :, CB:C], y_sb[:])

            def body(_=None):
                for t in range(n_ct):
                    bf16_tile(t, mm2_fp8=(q3_mm2 and fp8 and t == n_ct - 1))
                if fp8:
                    fp8_tile()

            if n_iters == 1:
                body()
            else:
                with tc.For_i(0, n_iters, 1):
                    body()
    nc.compile()
    return nc


def _get_nc():
    global _NC
    if _NC is None:
        _NC = _build_nc()
    return _NC


def _route(x, wr):
    """Replicates the reference router exactly (fp32 numpy)."""
    xt = np.transpose(x, (1, 0, 2)).reshape(T, D)  # [T, D] fp32
    logits = xt.astype(np.float32) @ wr.astype(np.float32)  # [T, E]
    m = logits.max(axis=-1, keepdims=True)
    p = np.exp(logits - m, dtype=np.float32)
    p /= p.sum(axis=-1, keepdims=True)
    top1 = np.argmax(p, axis=-1)
    p_masked = p.copy()
    p_masked[np.arange(T), top1] = -np.inf
    top2 = np.argmax(p_masked, axis=-1)
    eidx = np.stack([top1, top2], axis=1)  # [T, K]
    ew = np.take_along_axis(p, eidx, axis=1).astype(np.float32)  # [T, K]

    flat_e = eidx.reshape(-1)
    order = np.argsort(flat_e, kind="stable")
    sorted_e = flat_e[order]
    hist = np.bincount(flat_e, minlength=E)
    starts = np.cumsum(hist) - hist
    pos = np.arange(T * K) - starts[sorted_e]
    keep = pos < C
    slot = np.where(keep, sorted_e * C + pos, E * C)
    token = order // K
    return xt, ew, order, keep, slot, token


def _make_in_maps(x, wr, w1, w2, fp8=True):
    xt, ew, order, keep, slot, token = _route(x, wr)
    slot_token = np.zeros(E * C, np.int64)
    slot_token[slot[keep]] = token[keep]
    # routing weight per slot (0 for unfilled capacity slots)
    ew_flat = ew.reshape(-1)
    w_sorted = ew_flat[order]
    slot_ew = np.zeros(E * C, np.float32)
    slot_ew[slot[keep]] = w_sorted[keep]

    xT = np.ascontiguousarray(xt.T)  # [D, T] fp32
    in_maps = []
    perms = []
    for e in range(E):
        we = slot_ew[e * C : (e + 1) * C]
        if fp8:
            # descending routing weight; Q smallest-weight slots go fp8
            perm = np.argsort(-we, kind="stable")
        else:
            perm = np.arange(C)
        perms.append(perm)
        idx = slot_token[e * C + perm]
        m = {
            "xgT": np.ascontiguousarray(xT[:, idx[:CB if fp8 else C]]).astype(BF16),
            "w1": np.ascontiguousarray(w1[e].astype(BF16)),
            "w2": np.ascontiguousarray(w2[e].astype(BF16)),
        }
        if fp8:
            m["xg8T"] = np.ascontiguousarray(
                (xT[:, idx[CB:]] * SX)).astype(F8)
            m["w18"] = np.ascontiguousarray((w1[e] * SW)).astype(F8)
            m["w28"] = np.ascontiguousarray((w2[e] * SW)).astype(F8)
        in_maps.append(m)
    return in_maps, (ew, order, keep, slot, perms)


def _combine(Y_bufs, meta):
    """Y_bufs: [E*C, D] in permuted (per-expert) buffer order."""
    ew, order, keep, slot, perms = meta
    Y = np.empty((E * C, D), np.float32)
    for e in range(E):
        Y[e * C + perms[e]] = Y_bufs[e * C : (e + 1) * C]

    inv = np.empty(T * K, np.int64)
    inv[order] = np.arange(T * K)
    slot_tk = slot[inv].reshape(T, K)
    keep_tk = keep[inv].reshape(T, K)

    out_flat = np.zeros((T, D), np.float32)
    for k in range(K):
        sl = np.clip(slot_tk[:, k], 0, E * C - 1)
        contrib = Y[sl] * ew[:, k : k + 1]
        contrib[~keep_tk[:, k]] = 0.0
        out_flat += contrib
    return np.ascontiguousarray(
        out_flat.reshape(S, B, D).transpose(1, 0, 2)
    ).astype(np.float32)


def kernel(x, wr, w1, w2):
    nc = _get_nc()
    in_maps, meta = _make_in_maps(x, wr, w1, w2)

    res = run_bass_kernel_spmd(nc, in_maps, core_ids=list(range(N_CORES)))

    Y = np.empty((E * C, D), np.float32)
    for e in range(E):
        Y[e * C : (e + 1) * C] = res.results[e]["yT"].T.astype(np.float32)

    return _combine(Y, meta)


# ---------------------------------------------------------------------------
# Benchmark helper (used by test.py; not part of the grading contract).
# ---------------------------------------------------------------------------


def make_bench(in_maps, nc=None):
    import jax
    from jax.experimental.shard_map import shard_map
    from jax.sharding import Mesh, PartitionSpec, NamedSharding
    from concourse.bass2jax import (
        _bass_exec_p,
        install_neuronx_cc_hook,
        partition_id_tensor,
    )

    if nc is None:
        nc = _NC if _NC is not None else _get_nc()
    install_neuronx_cc_hook()
    partition_name = nc.partition_id_tensor.name if nc.partition_id_tensor else None

    in_names, out_names, out_avals, zero_outs = [], [], [], []
    for alloc in nc.m.functions[0].allocations:
        if not isinstance(alloc, mybir.MemoryLocationSet):
            continue
        name = alloc.memorylocations[0].name
        if alloc.kind == "ExternalInput":
            if name != partition_name:
                in_names.append(name)
        elif alloc.kind == "ExternalOutput":
            shape = tuple(alloc.tensor_shape)
            dtype = mybir.dt.np(alloc.dtype)
            out_avals.append(jax.core.ShapedArray(shape, dtype))
            zero_outs.append(np.zeros(shape, dtype))
            out_names.append(name)
    n_params = len(in_names)
    all_in_names = list(in_names) + list(out_names)
    if partition_name is not None:
        all_in_names.append(partition_name)
    if nc.dbg_addr is not None:
        dbg_zero = np.zeros((1, 2), np.uint32)
        in_maps = [{**m, nc.dbg_addr.name: dbg_zero} for m in in_maps]

    def _body(*args):
        operands = list(args)
        if partition_name is not None:
            operands.append(partition_id_tensor())
        outs = _bass_exec_p.bind(
            *operands,
            out_avals=tuple(out_avals),
            in_names=tuple(all_in_names),
            out_names=tuple(out_names),
            lowering_input_output_aliases=(),
            sim_require_finite=True,
            sim_require_nnan=True,
            nc=nc,
        )
        return tuple(outs)

    devices = jax.devices()[:N_CORES]
    mesh = Mesh(np.asarray(devices), ("core",))
    n_outs = len(out_names)
    in_specs = (PartitionSpec("core"),) * (n_params + n_outs)
    out_specs = (PartitionSpec("core"),) * n_outs
    fn = jax.jit(
        shard_map(_body, mesh=mesh, in_specs=in_specs, out_specs=out_specs,
                  check_rep=False),
        keep_unused=True,
    )
    concat_in = [
        np.concatenate([np.asarray(in_maps[c][name]) for c in range(N_CORES)],
                       axis=0)
        for name in in_names
    ]
    concat_zeros = [
        np.zeros((N_CORES * z.shape[0], *z.shape[1:]), z.dtype)
        for z in zero_outs
    ]
    shard = NamedSharding(mesh, PartitionSpec("core"))
    args = [jax.device_put(a, shard) for a in concat_in + concat_zeros]
    return fn, args, out_names


def benchmark(in_maps, iters=20, warmup=3, nc=None):
    import time
    import jax

    fn, args, out_names = make_bench(in_maps, nc=nc)
    for _ in range(warmup):
        out = fn(*args)
        jax.block_until_ready(out)
    times = []
    for _ in range(iters):
        t0 = time.perf_counter()
        out = fn(*args)
        jax.block_until_ready(out)
        times.append(time.perf_counter() - t0)
    return min(times), sorted(times)[len(times) // 2], out


# revision 9
# speedup vs baseline: 2.2957x; 1.1055x over previous
"""MegablockMoE kernel for 8 Trainium2 NeuronCores.

Strategy (per sharding hint): expert-parallel. The router + token
dispatch/combine permutations (pure index bookkeeping) run on host as the
shard/unshard step; each of the 8 cores owns one expert and runs the two big
GEMMs (gelu(xg @ w1[e]) @ w2[e], 34.4 GFLOP/core) with weights resident in
SBUF and the hT intermediate never leaving chip.

Precision split: the kernel is PE-bound at the bf16 roofline, so the slots
with the smallest routing weights (per expert) run partially in fp8-e4m3
with DoubleRow matmuls (2 fp8 MACs/cell/cycle). Host sorts each expert's
capacity buffer by routing weight (descending): cols 0..1023 run bf16,
cols 1024..1535 (3rd quartile) run bf16 mm1 + fp8 mm2, cols 1536..2047
(smallest weights) run fully in fp8. The fp8 error is attenuated by the
small routing weights: measured end-to-end rel-L2 1.73e-2 vs the 2e-2 gate
(deterministic; HW tracks the numpy prediction within 1e-4).

Device kernel (identical NEFF on all 8 cores, SPMD over experts):
    in : xgT  [D, 1536] bf16, xg8T [D, 512] fp8 (x16)
         w1/w2 [.,.] bf16, w18/w28 fp8 (x4096, streamed)
    out: yT [D, C] bf16
"""

import numpy as np
import ml_dtypes

import concourse.mybir as mybir
import concourse.tile as tile
from concourse import bacc
from concourse.bass_utils import run_bass_kernel_spmd

B, S, D = 4, 2048, 1024
E, K, DFF = 8, 2, 4096
T = B * S
C = K * T // E  # 2048 expert capacity
BF16 = ml_dtypes.bfloat16
F8 = ml_dtypes.float8_e4m3  # == mybir.dt.np(float8e4)
N_CORES = 8

Q = 512           # fp8 slots per expert (smallest routing weight)
CB = C - Q        # bf16 slots
SX = 16.0         # fp8 scale for activations x
SW = 4096.0       # fp8 scale for weights

KO1, KO2 = D // 128, DFF // 128
W1_CH = 8            # f-chunks of w1 (separate tiles -> fine-grained DMA deps)
W1_F = DFF // W1_CH  # 512
W2_CH = 8            # o-chunks of w2
W2_O = KO2 // W2_CH  # 4

_NC = None


def _build_nc(c_tile=512, ps1_bufs=4, ps2_bufs=4, ht_bufs=1, y_bufs=2,
              xg_bufs=2, y_dt="bf16", fp8=True, q3_mm2=True, n_iters=1,
              debug=True):
    f32 = mybir.dt.float32
    f8 = mybir.dt.float8e4
    y_mydt = mybir.dt.bfloat16 if y_dt == "bf16" else mybir.dt.float32
    DR = mybir.MatmulPerfMode.DoubleRow
    cb = CB if fp8 else C

    nc = bacc.Bacc(None, target_bir_lowering=False, debug=debug)
    xgT = nc.dram_tensor("xgT", [D, cb], mybir.dt.bfloat16, kind="ExternalInput")
    w1 = nc.dram_tensor("w1", [D, DFF], mybir.dt.bfloat16, kind="ExternalInput")
    w2 = nc.dram_tensor("w2", [DFF, D], mybir.dt.bfloat16, kind="ExternalInput")
    if fp8:
        xg8T = nc.dram_tensor("xg8T", [D, Q], f8, kind="ExternalInput")
        w18 = nc.dram_tensor("w18", [D, DFF], f8, kind="ExternalInput")
        w28 = nc.dram_tensor("w28", [DFF, D], f8, kind="ExternalInput")
    yT = nc.dram_tensor("yT", [D, C], y_mydt, kind="ExternalOutput")

    xgT_v = xgT.rearrange("(o p) c -> p o c", p=128)
    w1_v = w1.rearrange("(o p) f -> p o f", p=128)
    w2_v = w2.rearrange("(o p) d -> p o d", p=128)
    yT_v = yT.rearrange("(o p) c -> p o c", p=128)
    if fp8:
        xg8T_v = xg8T.rearrange("(o p) c -> p o c", p=128)
        w18_v = w18.rearrange("(o2 i p) f -> p o2 i f", p=128, i=2)
        w28_v = w28.rearrange("(f2 i p) d -> p f2 i d", p=128, i=2)
    n_ct = cb // c_tile

    with tile.TileContext(nc) as tc:
        with (
            tc.tile_pool(name="wpool", bufs=1) as wpool,
            tc.tile_pool(name="w18p", bufs=4) as w18p,
            tc.tile_pool(name="w28p", bufs=4) as w28p,
            tc.tile_pool(name="xpool", bufs=xg_bufs) as xpool,
            tc.tile_pool(name="hpool", bufs=ht_bufs) as hpool,
            tc.tile_pool(name="ypool", bufs=y_bufs) as ypool,
            tc.tile_pool(name="ps1", bufs=ps1_bufs, space="PSUM") as ps1,
            tc.tile_pool(name="ps2", bufs=ps2_bufs, space="PSUM") as ps2,
        ):
            # first xg tile before weights: small and needed immediately
            xg_tiles = {}
            if n_iters == 1:
                xg_tiles[0] = xpool.tile([128, KO1, c_tile], mybir.dt.bfloat16,
                                         tag="xg", name="xg0")
                nc.sync.dma_start(xg_tiles[0][:], xgT_v[:, :, 0:c_tile])

            w1_tiles = []
            for ch in range(W1_CH):
                wt = wpool.tile([128, KO1, W1_F], mybir.dt.bfloat16,
                                tag=f"w1_{ch}", name=f"w1t{ch}")
                nc.sync.dma_start(wt[:], w1_v[:, :, ch * W1_F : (ch + 1) * W1_F])
                w1_tiles.append(wt)
            w2_tiles = []
            for ch in range(W2_CH):
                wt = wpool.tile([128, W2_O, D], mybir.dt.bfloat16,
                                tag=f"w2_{ch}", name=f"w2t{ch}")
                nc.sync.dma_start(wt[:], w2_v[:, ch * W2_O : (ch + 1) * W2_O, :])
                w2_tiles.append(wt)

            def w1_ap(o, f):
                ch, r = divmod(f, W1_F // 128)
                return w1_tiles[ch][:, o, r * 128 : (r + 1) * 128]

            def w2_ap(f, g):
                ch, r = divmod(f, W2_O)
                return w2_tiles[ch][:, r, g * 128 : (g + 1) * 128]

            def evict_y(y_sb, g, psum, scale=None):
                # split evictions across DVE and ACT so neither engine's
                # latency throttles the PE stream
                if scale is None:
                    if g % 2 == 1:
                        nc.scalar.copy(y_sb[:, g, :], psum[:])
                    else:
                        nc.vector.tensor_copy(y_sb[:, g, :], psum[:])
                else:
                    if g % 2 == 1:
                        nc.scalar.activation(
                            y_sb[:, g, :], psum[:],
                            mybir.ActivationFunctionType.Copy, scale=scale)
                    else:
                        nc.vector.tensor_scalar_mul(y_sb[:, g, :], psum[:],
                                                    scale)

            def get_w28(pool_cache, it, f2, half):
                key = (f2, half)
                if key not in pool_cache:
                    wt = w28p.tile([128, 2, 512], f8, tag="w28",
                                   name=f"w28c{it}_{f2}_{half}")
                    nc.sync.dma_start(
                        wt[:],
                        w28_v[:, f2, :, half * 512 : (half + 1) * 512])
                    pool_cache[key] = wt
                return pool_cache[key]

            def bf16_tile(t, mm2_fp8=False):
                cs = slice(t * c_tile, (t + 1) * c_tile)
                if t not in xg_tiles:
                    xg_tiles[t] = xpool.tile(
                        [128, KO1, c_tile], mybir.dt.bfloat16, tag="xg",
                        name=f"xg{t}",
                    )
                    nc.sync.dma_start(xg_tiles[t][:], xgT_v[:, :, cs])
                xg_sb = xg_tiles[t]

                h_dt = f8 if mm2_fp8 else mybir.dt.bfloat16
                hT_sb = hpool.tile([128, KO2, c_tile], h_dt, tag="hT",
                                   name=f"hT{t}")
                y_sb = ypool.tile([128, KO1, c_tile], y_mydt, tag="y",
                                  name=f"y{t}")
                w28c = {}
                ev_scale = (1.0 / SW) if mm2_fp8 else None

                def mm2_step(accums, f2, g, half):
                    w28t = get_w28(w28c, f"q{t}", f2, half)
                    nc.tensor.matmul(
                        accums[g][:], w28t[:, :, g * 128 : (g + 1) * 128],
                        hT_sb[:, 2 * f2 : 2 * f2 + 2, :],
                        start=(f2 == 0), stop=(f2 == KO2 // 2 - 1),
                        perf_mode=DR,
                    )

                # pass A: mm1(f) interleaved with mm2 for g in 0..3 —
                # mm2's f-step consumes hT[f] right after its eviction,
                # removing the mm1->mm2 phase boundary.
                ps2g = [ps2.tile([128, c_tile], f32, tag="p2",
                                 name=f"p2a{t}_{g}")
                        for g in range(4)]
                for f in range(KO2):
                    psum = ps1.tile([128, c_tile], f32, tag="p1")
                    for o in range(KO1):
                        nc.tensor.matmul(
                            psum[:], w1_ap(o, f), xg_sb[:, o, :],
                            start=(o == 0), stop=(o == KO1 - 1),
                        )
                    nc.scalar.activation(
                        hT_sb[:, f, :], psum[:],
                        mybir.ActivationFunctionType.Gelu,
                    )
                    if mm2_fp8:
                        if f % 2 == 1:
                            for g in range(4):
                                mm2_step(ps2g, f // 2, g, 0)
                    else:
                        for g in range(4):
                            nc.tensor.matmul(
                                ps2g[g][:], w2_ap(f, g), hT_sb[:, f, :],
                                start=(f == 0), stop=(f == KO2 - 1),
                            )
                for g in range(4):
                    evict_y(y_sb, g, ps2g[g], scale=ev_scale)
                # pass B: mm2 for g in 4..7 (hT complete by now)
                ps2h = [ps2.tile([128, c_tile], f32, tag="p2",
                                 name=f"p2b{t}_{g}")
                        for g in range(4)]
                if mm2_fp8:
                    for f2 in range(KO2 // 2):
                        w28t = get_w28(w28c, f"q{t}", f2, 1)
                        for g in range(4):
                            nc.tensor.matmul(
                                ps2h[g][:],
                                w28t[:, :, g * 128 : (g + 1) * 128],
                                hT_sb[:, 2 * f2 : 2 * f2 + 2, :],
                                start=(f2 == 0), stop=(f2 == KO2 // 2 - 1),
                                perf_mode=DR,
                            )
                else:
                    for f in range(KO2):
                        for g in range(4):
                            nc.tensor.matmul(
                                ps2h[g][:], w2_ap(f, g + 4), hT_sb[:, f, :],
                                start=(f == 0), stop=(f == KO2 - 1),
                            )
                for g in range(4):
                    evict_y(y_sb, g + 4, ps2h[g], scale=ev_scale)
                nc.sync.dma_start(yT_v[:, :, cs], y_sb[:])

            def fp8_tile(it=""):
                # slots CB..C-1 in fp8 with DoubleRow (256-contraction MMs)
                xg8_sb = xpool.tile([128, KO1, Q], f8, tag="xg",
                                    name=f"xg8{it}")
                nc.sync.dma_start(xg8_sb[:], xg8T_v[:, :, :])
                h8_sb = hpool.tile([128, KO2, Q], f8, tag="hT",
                                   name=f"h8{it}")
                y_sb = ypool.tile([128, KO1, Q], y_mydt, tag="y",
                                  name=f"y8{it}")
                w18c = {}
                w28c = {}

                def get_w18(f):
                    if f not in w18c:
                        wt = w18p.tile([128, 4, 2, 128], f8, tag="w18",
                                       name=f"w18c{it}_{f}")
                        nc.sync.dma_start(
                            wt[:], w18_v[:, :, :, f * 128 : (f + 1) * 128])
                        w18c[f] = wt
                    return w18c[f]

                def get_w28(f2, half):
                    key = (f2, half)
                    if key not in w28c:
                        wt = w28p.tile([128, 2, 512], f8, tag="w28",
                                       name=f"w28c{it}_{f2}_{half}")
                        nc.sync.dma_start(
                            wt[:],
                            w28_v[:, f2, :, half * 512 : (half + 1) * 512])
                        w28c[key] = wt
                    return w28c[key]

                ps2g = [ps2.tile([128, Q], f32, tag="p2", name=f"p2f8a{it}_{g}")
                        for g in range(4)]
                for f in range(KO2):
                    psum = ps1.tile([128, Q], f32, tag="p1")
                    w18t = get_w18(f)
                    for o2 in range(4):
                        nc.tensor.matmul(
                            psum[:], w18t[:, o2, :, :],
                            xg8_sb[:, 2 * o2 : 2 * o2 + 2, :],
                            start=(o2 == 0), stop=(o2 == 3), perf_mode=DR,
                        )
                    nc.scalar.activation(
                        h8_sb[:, f, :], psum[:],
                        mybir.ActivationFunctionType.Gelu,
                        scale=1.0 / (SX * SW),
                    )
                    if f % 2 == 1:
                        f2 = f // 2
                        w28t = get_w28(f2, 0)
                        for g in range(4):
                            nc.tensor.matmul(
                                ps2g[g][:], w28t[:, :, g * 128 : (g + 1) * 128],
                                h8_sb[:, 2 * f2 : 2 * f2 + 2, :],
                                start=(f2 == 0), stop=(f2 == 15), perf_mode=DR,
                            )
                for g in range(4):
                    evict_y(y_sb, g, ps2g[g], scale=1.0 / SW)
                ps2h = [ps2.tile([128, Q], f32, tag="p2", name=f"p2f8b{it}_{g}")
                        for g in range(4)]
                for f2 in range(KO2 // 2):
                    w28t = get_w28(f2, 1)
                    for g in range(4):
                        nc.tensor.matmul(
                            ps2h[g][:], w28t[:, :, g * 128 : (g + 1) * 128],
                            h8_sb[:, 2 * f2 : 2 * f2 + 2, :],
                            start=(f2 == 0), stop=(f2 == 15), perf_mode=DR,
                        )
                for g in range(4):
                    evict_y(y_sb, g + 4, ps2h[g], scale=1.0 / SW)
                nc.sync.dma_start(yT_v[:, :, CB:C], y_sb[:])

            def body(_=None):
                for t in range(n_ct):
                    bf16_tile(t, mm2_fp8=(q3_mm2 and fp8 and t == n_ct - 1))
                if fp8:
                    fp8_tile()

            if n_iters == 1:
                body()
            else:
                with tc.For_i(0, n_iters, 1):
                    body()
    nc.compile()
    return nc


def _get_nc():
    global _NC
    if _NC is None:
        _NC = _build_nc()
    return _NC


def _route(x, wr):
    """Replicates the reference router exactly (fp32 numpy)."""
    xt = np.transpose(x, (1, 0, 2)).reshape(T, D)  # [T, D] fp32
    logits = xt.astype(np.float32) @ wr.astype(np.float32)  # [T, E]
    m = logits.max(axis=-1, keepdims=True)
    p = np.exp(logits - m, dtype=np.float32)
    p /= p.sum(axis=-1, keepdims=True)
    top1 = np.argmax(p, axis=-1)
    p_masked = p.copy()
    p_masked[np.arange(T), top1] = -np.inf
    top2 = np.argmax(p_masked, axis=-1)
    eidx = np.stack([top1, top2], axis=1)  # [T, K]
    ew = np.take_along_axis(p, eidx, axis=1).astype(np.float32)  # [T, K]

    flat_e = eidx.reshape(-1)
    order = np.argsort(flat_e, kind="stable")
    sorted_e = flat_e[order]
    hist = np.bincount(flat_e, minlength=E)
    starts = np.cumsum(hist) - hist
    pos = np.arange(T * K) - starts[sorted_e]
    keep = pos < C
    slot = np.where(keep, sorted_e * C + pos, E * C)
    token = order // K
    return xt, ew, order, keep, slot, token


def _make_in_maps(x, wr, w1, w2, fp8=True):
    xt, ew, order, keep, slot, token = _route(x, wr)
    slot_token = np.zeros(E * C, np.int64)
    slot_token[slot[keep]] = token[keep]
    # routing weight per slot (0 for unfilled capacity slots)
    ew_flat = ew.reshape(-1)
    w_sorted = ew_flat[order]
    slot_ew = np.zeros(E * C, np.float32)
    slot_ew[slot[keep]] = w_sorted[keep]

    xT = np.ascontiguousarray(xt.T)  # [D, T] fp32
    in_maps = []
    perms = []
    for e in range(E):
        we = slot_ew[e * C : (e + 1) * C]
        if fp8:
            # descending routing weight; Q smallest-weight slots go fp8
            perm = np.argsort(-we, kind="stable")
        else:
            perm = np.arange(C)
        perms.append(perm)
        idx = slot_token[e * C + perm]
        m = {
            "xgT": np.ascontiguousarray(xT[:, idx[:CB if fp8 else C]]).astype(BF16),
            "w1": np.ascontiguousarray(w1[e].astype(BF16)),
            "w2": np.ascontiguousarray(w2[e].astype(BF16)),
        }
        if fp8:
            m["xg8T"] = np.ascontiguousarray(
                (xT[:, idx[CB:]] * SX)).astype(F8)
            m["w18"] = np.ascontiguousarray((w1[e] * SW)).astype(F8)
            m["w28"] = np.ascontiguousarray((w2[e] * SW)).astype(F8)
        in_maps.append(m)
    return in_maps, (ew, order, keep, slot, perms)


def _combine(Y_bufs, meta):
    """Y_bufs: [E*C, D] in permuted (per-expert) buffer order."""
    ew, order, keep, slot, perms = meta
    Y = np.empty((E * C, D), np.float32)
    for e in range(E):
        Y[e * C + perms[e]] = Y_bufs[e * C : (e + 1) * C]

    inv = np.empty(T * K, np.int64)
    inv[order] = np.arange(T * K)
    slot_tk = slot[inv].reshape(T, K)
    keep_tk = keep[inv].reshape(T, K)

    out_flat = np.zeros((T, D), np.float32)
    for k in range(K):
        sl = np.clip(slot_tk[:, k], 0, E * C - 1)
        contrib = Y[sl] * ew[:, k : k + 1]
        contrib[~keep_tk[:, k]] = 0.0
        out_flat += contrib
    return np.ascontiguousarray(
        out_flat.reshape(S, B, D).transpose(1, 0, 2)
    ).astype(np.float32)


def kernel(x, wr, w1, w2):
    nc = _get_nc()
    in_maps, meta = _make_in_maps(x, wr, w1, w2)

    res = run_bass_kernel_spmd(nc, in_maps, core_ids=list(range(N_CORES)))

    Y = np.empty((E * C, D), np.float32)
    for e in range(E):
        Y[e * C : (e + 1) * C] = res.results[e]["yT"].T.astype(np.float32)

    return _combine(Y, meta)


# ---------------------------------------------------------------------------
# Benchmark helper (used by test.py; not part of the grading contract).
# ---------------------------------------------------------------------------


def make_bench(in_maps, nc=None):
    import jax
    from jax.experimental.shard_map import shard_map
    from jax.sharding import Mesh, PartitionSpec, NamedSharding
    from concourse.bass2jax import (
        _bass_exec_p,
        install_neuronx_cc_hook,
        partition_id_tensor,
    )

    if nc is None:
        nc = _NC if _NC is not None else _get_nc()
    install_neuronx_cc_hook()
    partition_name = nc.partition_id_tensor.name if nc.partition_id_tensor else None

    in_names, out_names, out_avals, zero_outs = [], [], [], []
    for alloc in nc.m.functions[0].allocations:
        if not isinstance(alloc, mybir.MemoryLocationSet):
            continue
        name = alloc.memorylocations[0].name
        if alloc.kind == "ExternalInput":
            if name != partition_name:
                in_names.append(name)
        elif alloc.kind == "ExternalOutput":
            shape = tuple(alloc.tensor_shape)
            dtype = mybir.dt.np(alloc.dtype)
            out_avals.append(jax.core.ShapedArray(shape, dtype))
            zero_outs.append(np.zeros(shape, dtype))
            out_names.append(name)
    n_params = len(in_names)
    all_in_names = list(in_names) + list(out_names)
    if partition_name is not None:
        all_in_names.append(partition_name)
    if nc.dbg_addr is not None:
        dbg_zero = np.zeros((1, 2), np.uint32)
        in_maps = [{**m, nc.dbg_addr.name: dbg_zero} for m in in_maps]

    def _body(*args):
        operands = list(args)
        if partition_name is not None:
            operands.append(partition_id_tensor())
        outs = _bass_exec_p.bind(
            *operands,
            out_avals=tuple(out_avals),
            in_names=tuple(all_in_names),
            out_names=tuple(out_names),
            lowering_input_output_aliases=(),
            sim_require_finite=True,
            sim_require_nnan=True,
            nc=nc,
        )
        return tuple(outs)

    devices = jax.devices()[:N_CORES]
    mesh = Mesh(np.asarray(devices), ("core",))
    n_outs = len(out_names)
    in_specs = (PartitionSpec("core"),) * (n_params + n_outs)
    out_specs = (PartitionSpec("core"),) * n_outs
    fn = jax.jit(
        shard_map(_body, mesh=mesh, in_specs=in_specs, out_specs=out_specs,
                  check_rep=False),
        keep_unused=True,
    )
    concat_in = [
        np.concatenate([np.asarray(in_maps[c][name]) for c in range(N_CORES)],
                       axis=0)
        for name in in_names
    ]
    concat_zeros = [
        np.zeros((N_CORES * z.shape[0], *z.shape[1:]), z.dtype)
        for z in zero_outs
    ]
    shard = NamedSharding(mesh, PartitionSpec("core"))
    args = [jax.device_put(a, shard) for a in concat_in + concat_zeros]
    return fn, args, out_names


def benchmark(in_maps, iters=20, warmup=3, nc=None):
    import time
    import jax

    fn, args, out_names = make_bench(in_maps, nc=nc)
    for _ in range(warmup):
        out = fn(*args)
        jax.block_until_ready(out)
    times = []
    for _ in range(iters):
        t0 = time.perf_counter()
        out = fn(*args)
        jax.block_until_ready(out)
        times.append(time.perf_counter() - t0)
    return min(times), sorted(times)[len(times) // 2], out
